# revision 18
# baseline (speedup 1.0000x reference)
"""Trainium2 Bass kernel for nn_EnhancedS4Layer.

Math: the S4 FFT long-conv kernel k[f,d] = dt[f] * sum_n B[n,f] C[f,n] mix[n] r_n^d
with r_n = exp(-|A_real[n]|) <= 0.875, so k decays below 4e-8 by lag 128: the conv
is exactly (to fp32 noise) a 128-tap depthwise FIR. Each channel's FIR is applied
as two 128x128 Toeplitz matmuls per 128-sample chunk (current chunk + previous
chunk), with the per-channel Toeplitz matrices as the PE stationary operand and
all (batch, chunk) instances streamed as the moving operand.

Launch 1 (channel-sharded, 64 ch/core x all 8 batches): the FIR conv with fp16
operands (full PE rate, half HBM traffic). The D*x skip is folded into tap
k[f,0]; backward (anticausal) channels are handled by host-side time reversal
of x (and of y after), exactly mirroring the reference's flip-conv-flip.
Output is streamed back as fp16 in a partition-major layout.

Launch 2 (batch-sharded, 1 batch/core, partition-major [p, t, f] layout):
streamed LayerNorm+Gelu — per tile-group: bn_stats/bn_aggr on vector,
rsqrt(var+eps) via bit-trick + 2 Newton steps on vector (no scalar-engine
Sqrt table load), then a single fused scalar-engine
Gelu(y * rsqrt + (-mu*rsqrt)) per tile using per-partition scale/bias APs.
Loads, vector stats, scalar gelu and stores pipeline; no phase barrier.

Host does only layout work (transpose/pad/flip) and O(F*N*D) tap precompute.
"""
import numpy as np

import concourse.bacc as bacc
import concourse.tile as tile
from concourse import mybir
from concourse.bass_utils import run_bass_kernel_spmd

BATCH, F, L, N = 8, 512, 8192, 64
T = 128                    # chunk length == FIR tap count
TB = 64                    # previous-chunk contract rows (taps kept: lags 1..63)
C = L // T                 # 64 chunks per batch
NCORES = 8
CH = F // NCORES           # 64 channels per core in launch 1
GRP = 8                    # channels per SBUF-resident group in launch 1
SB = 4                     # channels per batched y store
BC = BATCH * C             # 512 moving columns per channel
NT = L // T                # 64 l-tiles per batch in launch 2
GT = 8                     # l-tiles per streamed group in launch 2
EPS = 1e-5
RSQRT_MAGIC = 0x5F3759DF

_programs = {}
LAST_EXEC_NS = {}

# precision knobs (fp16 halves HBM traffic for the respective stream)
import os as _os
Y_FP16 = _os.environ.get("S4_Y_FP16", "1") == "1"   # conv→LN intermediate over HBM
X_FP16 = _os.environ.get("S4_X_FP16", "1") == "1"   # conv operands (x + Toeplitz wts)


def _build_l1():
    nc = bacc.Bacc()
    xdt = mybir.dt.float16 if X_FP16 else mybir.dt.float32r
    ydt = mybir.dt.float16 if Y_FP16 else mybir.dt.float32
    wts = nc.dram_tensor("wts", [T, CH, T], xdt, kind="ExternalInput")
    wtbs = nc.dram_tensor("wtbs", [TB, CH, TB], xdt, kind="ExternalInput")
    xt = nc.dram_tensor("xt", [T, CH, BATCH, C + 1], xdt, kind="ExternalInput")
    y = nc.dram_tensor("y", [T, CH, BC], ydt, kind="ExternalOutput")

    with tile.TileContext(nc) as tc:
        with tc.tile_pool(name="wp", bufs=3) as wp, \
             tc.tile_pool(name="xp", bufs=3) as xp, \
             tc.tile_pool(name="yp", bufs=4) as yp, \
             tc.tile_pool(name="ps", bufs=8, space="PSUM") as ps:
            for g in range(CH // GRP):
                wt = wp.tile([T, GRP, T], xdt, tag="wt")
                # stationary for the previous-chunk matmul must sit at base
                # partition 64 to match the moving operand's partition range
                wb = wp.tile([T, GRP, TB], xdt, tag="wb")
                xl = xp.tile([T, GRP, BATCH, C + 1], xdt, tag="xl")
                sl = slice(g * GRP, (g + 1) * GRP)
                nc.sync.dma_start(out=wb[T - TB:T, :, :], in_=wtbs[:, sl, :])
                if g == 0:
                    # fine-grained first loads: subtile deps let channel 0's
                    # matmuls start ~4x earlier than a whole-group load
                    for s in range(0, GRP, 2):
                        nc.sync.dma_start(out=wt[:, s:s + 2, :],
                                          in_=wts[:, s:s + 2, :])
                        nc.sync.dma_start(out=xl[:, s:s + 2, :, :],
                                          in_=xt[:, s:s + 2, :, :])
                else:
                    nc.sync.dma_start(out=wt, in_=wts[:, sl, :])
                    nc.sync.dma_start(out=xl, in_=xt[:, sl, :, :])
                yt = None
                for ci in range(GRP):
                    ch = g * GRP + ci
                    pt = ps.tile([T, BC], mybir.dt.float32, tag="pt")
                    # current chunk taps (lags 0..127), then previous chunk
                    # (taps truncated to lags 1..63, a ~5e-6 rel err: only
                    # contract partitions 65:128 against a [63,63] corner)
                    nc.tensor.matmul(pt, wt[:, ci, :], xl[:, ci, :, 1:1 + C],
                                     start=True, stop=False)
                    nc.tensor.matmul(pt[0:TB, :], wb[T - TB:T, ci, :],
                                     xl[T - TB:T, ci, :, 0:C],
                                     start=False, stop=True)
                    if ci % SB == 0:
                        yt = yp.tile([T, SB, BC], ydt, tag="yt")
                    if ci % 2 == 0:
                        nc.scalar.copy(out=yt[:, ci % SB, :], in_=pt[:])
                    else:
                        nc.vector.tensor_copy(out=yt[:, ci % SB, :], in_=pt[:])
                    if ci % SB == SB - 1:
                        # stores go out on the gpsimd queue so the in-order
                        # sync queue streams loads ahead without blocking
                        nc.gpsimd.dma_start(out=y[:, ch - SB + 1:ch + 1, :], in_=yt)
    nc.compile()
    return nc


def _build_l2(apply_w, apply_b):
    nc = bacc.Bacc()
    ydt = mybir.dt.float16 if Y_FP16 else mybir.dt.float32
    f32 = mybir.dt.float32
    yt = nc.dram_tensor("yt", [T, NT, F], ydt, kind="ExternalInput")
    out = nc.dram_tensor("out", [T, NT, F], f32, kind="ExternalOutput")
    if apply_w:
        wv = nc.dram_tensor("wv", [1, F], f32, kind="ExternalInput")
    if apply_b:
        bv = nc.dram_tensor("bv", [1, F], f32, kind="ExternalInput")

    with tile.TileContext(nc) as tc:
        with tc.tile_pool(name="dp", bufs=8) as dp, \
             tc.tile_pool(name="sp", bufs=8) as sp, \
             tc.tile_pool(name="vp", bufs=8) as vp, \
             tc.tile_pool(name="op", bufs=4) as op, \
             tc.tile_pool(name="cp", bufs=1) as cp:
            if apply_w:
                wt = cp.tile([T, F], f32, tag="wrep")
                nc.sync.dma_start(out=wt, in_=wv.to_broadcast([T, F]))
            if apply_b:
                bt = cp.tile([T, F], f32, tag="brep")
                nc.sync.dma_start(out=bt, in_=bv.to_broadcast([T, F]))
            for g in range(NT // GT):
                dt_ = dp.tile([T, GT, F], ydt, tag="d")
                if g == 0:
                    for s in range(0, GT, 2):
                        nc.sync.dma_start(out=dt_[:, s:s + 2, :],
                                          in_=yt[:, s:s + 2, :])
                else:
                    nc.sync.dma_start(out=dt_, in_=yt[:, g * GT:(g + 1) * GT, :])
                st = sp.tile([T, GT, 6], f32, tag="s")
                mv = sp.tile([T, GT, 2], f32, tag="mv")
                for k in range(GT):
                    nc.vector.bn_stats(out=st[:, k, :], in_=dt_[:, k, :])
                    nc.vector.bn_aggr(out=mv[:, k, :], in_=st[:, k, :])
                # rs = rsqrt(var + eps): bit-trick seed + 2 Newton steps, all
                # on the vector engine (keeps the scalar act table on Gelu)
                v = vp.tile([T, GT], f32, tag="v")
                rs = vp.tile([T, GT], f32, tag="rs")
                t1 = vp.tile([T, GT], f32, tag="t1")
                nb = vp.tile([T, GT], f32, tag="nb")
                nc.vector.tensor_scalar_add(out=v, in0=mv[:, :, 1], scalar1=EPS)
                vi = v[:].bitcast(mybir.dt.int32)
                rsi = rs[:].bitcast(mybir.dt.int32)
                nc.vector.tensor_scalar(out=rsi, in0=vi, scalar1=1, scalar2=None,
                                        op0=mybir.AluOpType.arith_shift_right)
                nc.vector.tensor_scalar(out=rsi, in0=rsi, scalar1=-1,
                                        scalar2=RSQRT_MAGIC,
                                        op0=mybir.AluOpType.mult,
                                        op1=mybir.AluOpType.add)
                for _ in range(2):
                    nc.vector.tensor_mul(out=t1, in0=v, in1=rs)
                    nc.vector.tensor_mul(out=t1, in0=t1, in1=rs)
                    nc.vector.tensor_scalar(out=t1, in0=t1, scalar1=-0.5,
                                            scalar2=1.5,
                                            op0=mybir.AluOpType.mult,
                                            op1=mybir.AluOpType.add)
                    nc.vector.tensor_mul(out=rs, in0=rs, in1=t1)
                nc.vector.scalar_tensor_tensor(out=nb, in0=mv[:, :, 0],
                                               scalar=-1.0, in1=rs,
                                               op0=mybir.AluOpType.mult,
                                               op1=mybir.AluOpType.mult)
                ot = op.tile([T, GT, F], f32, tag="o")
                for k in range(GT):
                    if not (apply_w or apply_b):
                        # out = Gelu(y*rs - mu*rs), per-partition scale/bias
                        nc.scalar.activation(out=ot[:, k, :], in_=dt_[:, k, :],
                                             func=mybir.ActivationFunctionType.Gelu,
                                             bias=nb[:, k:k + 1],
                                             scale=rs[:, k:k + 1])
                    else:
                        nc.vector.tensor_scalar(out=ot[:, k, :], in0=dt_[:, k, :],
                                                scalar1=mv[:, k, 0:1],
                                                scalar2=rs[:, k:k + 1],
                                                op0=mybir.AluOpType.subtract,
                                                op1=mybir.AluOpType.mult)
                        if apply_w:
                            nc.vector.tensor_mul(out=ot[:, k, :], in0=ot[:, k, :], in1=wt)
                        if apply_b:
                            nc.vector.tensor_add(out=ot[:, k, :], in0=ot[:, k, :], in1=bt)
                        nc.scalar.activation(out=ot[:, k, :], in_=ot[:, k, :],
                                             func=mybir.ActivationFunctionType.Gelu)
                nc.gpsimd.dma_start(out=out[:, g * GT:(g + 1) * GT, :], in_=ot)
    nc.compile()
    return nc


def _taps(A_real, B, C_, D, kernel_mix, log_dt):
    """k[f, d] for d in [0, T), with the D skip folded into lag 0."""
    r = np.exp(-np.abs(A_real.astype(np.float64)))            # [N]
    w = (B.astype(np.float64).T * C_.astype(np.float64)) \
        * kernel_mix.astype(np.float64)[None, :]              # [F, N]
    powers = r[:, None] ** np.arange(T)[None, :]              # [N, T]
    k = (w @ powers) * np.exp(log_dt.astype(np.float64))[:, None]  # [F, T]
    k[:, 0] += D.astype(np.float64)
    return k.astype(np.float32)


def _toeplitz_parts(k):
    """Per-channel stationary weights. ta [F, T, T]: current-chunk lower-band
    Toeplitz T_a[i,j]=k[j-i] (j>=i). tb [F, TB, TB]: previous-chunk corner
    (contract rows i=65..127, out cols m=0..62) with taps truncated to lags
    1..63: tb[r,m] = k[63+m-r] for m<=r."""
    i = np.arange(T)[:, None]
    j = np.arange(T)[None, :]
    lag_a = j - i                       # [T, T]
    ta = (k[:, np.clip(lag_a, 0, T - 1)] * (lag_a >= 0)[None]).astype(np.float32)
    r = np.arange(TB)[:, None]
    m = np.arange(TB)[None, :]
    lag_b = TB + m - r                  # in [1, 63] iff m < r
    tb = (k[:, np.clip(lag_b, 0, T - 1)]
          * ((lag_b >= 1) & (lag_b <= TB - 1))[None]).astype(np.float32)
    return ta, tb


def kernel(x, A_real, B, C_=None, D=None, kernel_mix=None, log_dt=None,
           ln_w=None, ln_b=None, **kw):
    # accept reference's exact names (C is shadowed by chunk-count above)
    if C_ is None:
        C_ = kw.pop("C")
    x = np.asarray(x, dtype=np.float32)
    A_real = np.asarray(A_real); B = np.asarray(B); C_ = np.asarray(C_)
    D = np.asarray(D); kernel_mix = np.asarray(kernel_mix)
    log_dt = np.asarray(log_dt); ln_w = np.asarray(ln_w); ln_b = np.asarray(ln_b)

    apply_w = not np.allclose(ln_w, 1.0)
    apply_b = not np.allclose(ln_b, 0.0)

    if "l1" not in _programs:
        _programs["l1"] = _build_l1()
    if ("l2", apply_w, apply_b) not in _programs:
        _programs[("l2", apply_w, apply_b)] = _build_l2(apply_w, apply_b)
    nc1 = _programs["l1"]
    nc2 = _programs[("l2", apply_w, apply_b)]

    # ---- host prep: taps + Toeplitz weights
    k = _taps(A_real, B, C_, D, kernel_mix, log_dt)       # [F, T]
    ta, tb = _toeplitz_parts(k)                            # [F, T, T], [F, TB, TB]

    # ---- host prep: flipped-x, transposed+padded moving operand
    xs = x.copy()
    xs[:, F // 2:, :] = xs[:, F // 2:, ::-1]              # anticausal -> causal
    # XT[i, f, b, 1+c] = xs[b, f, c*T + i]
    xr = np.ascontiguousarray(
        xs.reshape(BATCH, F, C, T).transpose(3, 1, 0, 2))  # [T, F, B, C]
    XT = np.zeros((T, F, BATCH, C + 1), dtype=np.float32)
    XT[:, :, :, 1:1 + C] = xr

    xdt_np = np.float16 if X_FP16 else np.float32
    ydt_np = np.float16 if Y_FP16 else np.float32
    in_maps1 = []
    for c in range(NCORES):
        sl = slice(c * CH, (c + 1) * CH)
        in_maps1.append({
            "wts": ta[sl].transpose(1, 0, 2).astype(xdt_np),   # [T, CH, T]
            "wtbs": tb[sl].transpose(1, 0, 2).astype(xdt_np),  # [TB, CH, TB]
            "xt": XT[:, sl].astype(xdt_np),                    # [T, CH, B, C+1]
        })
    r1 = run_bass_kernel_spmd(nc1, in_maps1, core_ids=list(range(NCORES)))
    LAST_EXEC_NS["l1"] = r1.exec_time_ns
    ys = np.stack([r1.results[c]["y"] for c in range(NCORES)])  # [8, T, CH, B*C]
    ys = ys.reshape(NCORES, T, CH, BATCH, C)

    # ---- host mid: un-flip backward channels (time l = c*T + i reverses in
    # both i and c), then assemble partition-major [B, p, t, F] for launch 2
    ys[NCORES // 2:] = ys[NCORES // 2:, ::-1, :, :, ::-1]
    # [core, i, ch, b, c] -> [b, p=i, t=c, f=(core, ch)]
    ytd = np.ascontiguousarray(ys.transpose(3, 1, 4, 0, 2)).reshape(BATCH, T, NT, F)

    in_maps2 = []
    for c in range(NCORES):
        m = {"yt": ytd[c]}
        if apply_w:
            m["wv"] = ln_w.astype(np.float32).reshape(1, F)
        if apply_b:
            m["bv"] = ln_b.astype(np.float32).reshape(1, F)
        in_maps2.append(m)
    r2 = run_bass_kernel_spmd(nc2, in_maps2, core_ids=list(range(NCORES)))
    LAST_EXEC_NS["l2"] = r2.exec_time_ns
    outd = np.stack([r2.results[c]["out"] for c in range(NCORES)])  # [B, p, t, F]
    # [b, p, t, f] -> [b, f, l=t*T+p]
    out = np.ascontiguousarray(outd.transpose(0, 3, 2, 1)).reshape(BATCH, F, L)
    return out


# revision 36
# speedup vs baseline: 1.2092x; 1.2092x over previous
"""Trainium2 Bass kernel for nn_EnhancedS4Layer.

Math: the S4 FFT long-conv kernel k[f,d] = dt[f] * sum_n B[n,f] C[f,n] mix[n] r_n^d
with r_n = exp(-|A_real[n]|) <= 0.875, so k decays below 4e-8 by lag 128: the conv
is exactly (to fp32 noise) a 128-tap depthwise FIR. Each channel's FIR is applied
as two 128x128 Toeplitz matmuls per 128-sample chunk (current chunk + previous
chunk), with the per-channel Toeplitz matrices as the PE stationary operand and
all (batch, chunk) instances streamed as the moving operand.

Launch 1 (channel-sharded, 64 ch/core x all 8 batches): the FIR conv with fp16
operands (full PE rate, half HBM traffic). The D*x skip is folded into tap
k[f,0]; backward (anticausal) channels are handled by host-side time reversal
of x (and of y after), exactly mirroring the reference's flip-conv-flip.
Output is streamed back as fp16 in a partition-major layout.

Launch 2 (batch-sharded, 1 batch/core, partition-major [p, t, f] layout):
streamed LayerNorm+Gelu — per tile-group: bn_stats/bn_aggr on vector,
rsqrt(var+eps) via bit-trick + 1 Newton step on vector (no scalar-engine
Sqrt table load), then a single fused scalar-engine
Gelu(y * rsqrt + (-mu*rsqrt)) per tile using per-partition scale/bias APs.
Loads, vector stats, scalar gelu and stores pipeline; no phase barrier.

Host does only layout work (transpose/pad/flip) and O(F*N*D) tap precompute.
"""
import numpy as np

import concourse.bacc as bacc
import concourse.tile as tile
from concourse import mybir
from concourse.bass_utils import run_bass_kernel_spmd

BATCH, F, L, N = 8, 512, 8192, 64
T = 128                    # chunk length == FIR tap count
C = L // T                 # 64 chunks per batch
NCORES = 8
CH = F // NCORES           # 64 channels per core in launch 1
GRP = 8                    # channels per SBUF-resident group in launch 1
SB = 4                     # channels per batched y store
BC = BATCH * C             # 512 moving columns per channel
NT = L // T                # 64 l-tiles per batch in launch 2
GT = 8                     # l-tiles per streamed group in launch 2
EPS = 1e-5
RSQRT_MAGIC = 0x5F3759DF

_programs = {}
LAST_EXEC_NS = {}

# precision knobs (fp16 halves HBM traffic for the respective stream)
import os as _os
Y_FP16 = _os.environ.get("S4_Y_FP16", "1") == "1"   # conv→LN intermediate over HBM
X_FP16 = _os.environ.get("S4_X_FP16", "1") == "1"   # conv operands (x + Toeplitz wts)
O_FP16 = _os.environ.get("S4_O_FP16", "1") == "1"   # gelu output over HBM (host casts to f32)


def _build_l1():
    nc = bacc.Bacc()
    xdt = mybir.dt.float16 if X_FP16 else mybir.dt.float32r
    ydt = mybir.dt.float16 if Y_FP16 else mybir.dt.float32
    wts = nc.dram_tensor("wts", [T, CH, 2 * T], xdt, kind="ExternalInput")
    xt = nc.dram_tensor("xt", [T, CH, BATCH, C + 1], xdt, kind="ExternalInput")
    y = nc.dram_tensor("y", [T, CH, BC], ydt, kind="ExternalOutput")

    with tile.TileContext(nc) as tc:
        with tc.tile_pool(name="wp", bufs=3) as wp, \
             tc.tile_pool(name="xp", bufs=3) as xp, \
             tc.tile_pool(name="yp", bufs=4) as yp, \
             tc.tile_pool(name="ps", bufs=8, space="PSUM") as ps:
            for g in range(CH // GRP):
                wt = wp.tile([T, GRP, 2 * T], xdt, tag="wt")
                xl = xp.tile([T, GRP, BATCH, C + 1], xdt, tag="xl")
                sl = slice(g * GRP, (g + 1) * GRP)
                if g == 0:
                    # fine-grained first loads: subtile deps let channel 0's
                    # matmuls start ~4x earlier than a whole-group load
                    for s in range(0, GRP, 2):
                        nc.sync.dma_start(out=wt[:, s:s + 2, :],
                                          in_=wts[:, s:s + 2, :])
                        nc.sync.dma_start(out=xl[:, s:s + 2, :, :],
                                          in_=xt[:, s:s + 2, :, :])
                else:
                    nc.sync.dma_start(out=wt, in_=wts[:, sl, :])
                    nc.sync.dma_start(out=xl, in_=xt[:, sl, :, :])
                yt = None
                for ci in range(GRP):
                    ch = g * GRP + ci
                    pt = ps.tile([T, BC], mybir.dt.float32, tag="pt")
                    # current chunk taps (lags 0..127), then previous chunk
                    # (lags 128+j-i folded as cols 0..C-1 == chunk c-1)
                    nc.tensor.matmul(pt, wt[:, ci, 0:T], xl[:, ci, :, 1:1 + C],
                                     start=True, stop=False)
                    nc.tensor.matmul(pt, wt[:, ci, T:2 * T], xl[:, ci, :, 0:C],
                                     start=False, stop=True)
                    if ci % SB == 0:
                        yt = yp.tile([T, SB, BC], ydt, tag="yt")
                    if ci % 2 == 0:
                        nc.scalar.copy(out=yt[:, ci % SB, :], in_=pt[:])
                    else:
                        nc.vector.tensor_copy(out=yt[:, ci % SB, :], in_=pt[:])
                    if ci % SB == SB - 1:
                        # stores go out on the gpsimd queue so the in-order
                        # sync queue streams loads ahead without blocking
                        nc.gpsimd.dma_start(out=y[:, ch - SB + 1:ch + 1, :], in_=yt)
    nc.compile()
    return nc


def _build_l2(apply_w, apply_b):
    nc = bacc.Bacc()
    ydt = mybir.dt.float16 if Y_FP16 else mybir.dt.float32
    odt = mybir.dt.float16 if O_FP16 else mybir.dt.float32
    f32 = mybir.dt.float32
    yt = nc.dram_tensor("yt", [T, NT, F], ydt, kind="ExternalInput")
    out = nc.dram_tensor("out", [T, NT, F], odt, kind="ExternalOutput")
    if apply_w:
        wv = nc.dram_tensor("wv", [1, F], f32, kind="ExternalInput")
    if apply_b:
        bv = nc.dram_tensor("bv", [1, F], f32, kind="ExternalInput")

    with tile.TileContext(nc) as tc:
        with tc.tile_pool(name="dp", bufs=8) as dp, \
             tc.tile_pool(name="sp", bufs=8) as sp, \
             tc.tile_pool(name="vp", bufs=8) as vp, \
             tc.tile_pool(name="op", bufs=4) as op, \
             tc.tile_pool(name="cp", bufs=1) as cp:
            if apply_w:
                wt = cp.tile([T, F], f32, tag="wrep")
                nc.sync.dma_start(out=wt, in_=wv.to_broadcast([T, F]))
            if apply_b:
                bt = cp.tile([T, F], f32, tag="brep")
                nc.sync.dma_start(out=bt, in_=bv.to_broadcast([T, F]))
            # ramped group sizes: tiny first groups get the scalar-engine gelu
            # stream (the serial bottleneck) started ~16us earlier
            sizes = [2, 2, 4] + [GT] * ((NT - 8) // GT)
            t0 = 0
            for g, sz in enumerate(sizes):
                dt_ = dp.tile([T, sz, F], ydt, tag=f"d{sz}")
                nc.sync.dma_start(out=dt_, in_=yt[:, t0:t0 + sz, :])
                st = sp.tile([T, sz, 6], f32, tag=f"s{sz}")
                mv = sp.tile([T, sz, 2], f32, tag=f"mv{sz}")
                for k in range(sz):
                    nc.vector.bn_stats(out=st[:, k, :], in_=dt_[:, k, :])
                    nc.vector.bn_aggr(out=mv[:, k, :], in_=st[:, k, :])
                # rs = rsqrt(var + eps): bit-trick seed + 1 Newton step, all
                # on the vector engine (keeps the scalar act table on Gelu)
                v = vp.tile([T, sz], f32, tag=f"v{sz}")
                rs = vp.tile([T, sz], f32, tag=f"rs{sz}")
                t1 = vp.tile([T, sz], f32, tag=f"t1{sz}")
                nb = vp.tile([T, sz], f32, tag=f"nb{sz}")
                nc.vector.tensor_scalar_add(out=v, in0=mv[:, :, 1], scalar1=EPS)
                vi = v[:].bitcast(mybir.dt.int32)
                rsi = rs[:].bitcast(mybir.dt.int32)
                nc.vector.tensor_scalar(out=rsi, in0=vi, scalar1=1, scalar2=None,
                                        op0=mybir.AluOpType.arith_shift_right)
                nc.vector.tensor_scalar(out=rsi, in0=rsi, scalar1=-1,
                                        scalar2=RSQRT_MAGIC,
                                        op0=mybir.AluOpType.mult,
                                        op1=mybir.AluOpType.add)
                # one Newton step: rs *= 1.5 - 0.5*v*rs^2 (max rel err ~1.8e-3
                # on rs; measured 1.6e-3 end-to-end vs the 2e-2 gate)
                nc.vector.tensor_mul(out=t1, in0=v, in1=rs)
                nc.vector.tensor_mul(out=t1, in0=t1, in1=rs)
                nc.vector.tensor_scalar(out=t1, in0=t1, scalar1=-0.5,
                                        scalar2=1.5,
                                        op0=mybir.AluOpType.mult,
                                        op1=mybir.AluOpType.add)
                nc.vector.tensor_mul(out=rs, in0=rs, in1=t1)
                nc.vector.scalar_tensor_tensor(out=nb, in0=mv[:, :, 0],
                                               scalar=-1.0, in1=rs,
                                               op0=mybir.AluOpType.mult,
                                               op1=mybir.AluOpType.mult)
                ot = op.tile([T, sz, F], odt, tag=f"o{sz}")
                for k in range(sz):
                    if not (apply_w or apply_b):
                        # out = Gelu(y*rs - mu*rs), per-partition scale/bias
                        nc.scalar.activation(out=ot[:, k, :], in_=dt_[:, k, :],
                                             func=mybir.ActivationFunctionType.Gelu,
                                             bias=nb[:, k:k + 1],
                                             scale=rs[:, k:k + 1])
                    else:
                        nc.vector.tensor_scalar(out=ot[:, k, :], in0=dt_[:, k, :],
                                                scalar1=mv[:, k, 0:1],
                                                scalar2=rs[:, k:k + 1],
                                                op0=mybir.AluOpType.subtract,
                                                op1=mybir.AluOpType.mult)
                        if apply_w:
                            nc.vector.tensor_mul(out=ot[:, k, :], in0=ot[:, k, :], in1=wt)
                        if apply_b:
                            nc.vector.tensor_add(out=ot[:, k, :], in0=ot[:, k, :], in1=bt)
                        nc.scalar.activation(out=ot[:, k, :], in_=ot[:, k, :],
                                             func=mybir.ActivationFunctionType.Gelu)
                nc.gpsimd.dma_start(out=out[:, t0:t0 + sz, :], in_=ot)
                t0 += sz
    nc.compile()
    return nc


def _taps(A_real, B, C_, D, kernel_mix, log_dt):
    """k[f, d] for d in [0, T), with the D skip folded into lag 0."""
    r = np.exp(-np.abs(A_real.astype(np.float64)))            # [N]
    w = (B.astype(np.float64).T * C_.astype(np.float64)) \
        * kernel_mix.astype(np.float64)[None, :]              # [F, N]
    powers = r[:, None] ** np.arange(T)[None, :]              # [N, T]
    k = (w @ powers) * np.exp(log_dt.astype(np.float64))[:, None]  # [F, T]
    k[:, 0] += D.astype(np.float64)
    return k.astype(np.float32)


def _toeplitz_pair(k):
    """Per-channel stationary weights [F, T, 2T]: cols 0:T = current-chunk
    lower-band Toeplitz T_a[i,j]=k[j-i] (j>=i); cols T:2T = previous-chunk
    T_b[i,j]=k[T+j-i] (i>j)."""
    i = np.arange(T)[:, None]
    j = np.arange(T)[None, :]
    lag_a = j - i                       # [T, T]
    lag_b = T + j - i
    mask_a = (lag_a >= 0)
    mask_b = (lag_b >= 1) & (lag_b < T)
    out = np.zeros((F, T, 2 * T), dtype=np.float32)
    out[:, :, 0:T] = k[:, np.clip(lag_a, 0, T - 1)] * mask_a[None]
    out[:, :, T:2 * T] = k[:, np.clip(lag_b, 0, T - 1)] * mask_b[None]
    return out


def kernel(x, A_real, B, C_=None, D=None, kernel_mix=None, log_dt=None,
           ln_w=None, ln_b=None, **kw):
    # accept reference's exact names (C is shadowed by chunk-count above)
    if C_ is None:
        C_ = kw.pop("C")
    x = np.asarray(x, dtype=np.float32)
    A_real = np.asarray(A_real); B = np.asarray(B); C_ = np.asarray(C_)
    D = np.asarray(D); kernel_mix = np.asarray(kernel_mix)
    log_dt = np.asarray(log_dt); ln_w = np.asarray(ln_w); ln_b = np.asarray(ln_b)

    apply_w = not np.allclose(ln_w, 1.0)
    apply_b = not np.allclose(ln_b, 0.0)

    if "l1" not in _programs:
        _programs["l1"] = _build_l1()
    if ("l2", apply_w, apply_b) not in _programs:
        _programs[("l2", apply_w, apply_b)] = _build_l2(apply_w, apply_b)
    nc1 = _programs["l1"]
    nc2 = _programs[("l2", apply_w, apply_b)]

    # ---- host prep: taps + Toeplitz weights
    k = _taps(A_real, B, C_, D, kernel_mix, log_dt)       # [F, T]
    tw = _toeplitz_pair(k)                                 # [F, T, 2T]

    # ---- host prep: flipped-x, transposed+padded moving operand
    xs = x.copy()
    xs[:, F // 2:, :] = xs[:, F // 2:, ::-1]              # anticausal -> causal
    # XT[i, f, b, 1+c] = xs[b, f, c*T + i]
    xr = np.ascontiguousarray(
        xs.reshape(BATCH, F, C, T).transpose(3, 1, 0, 2))  # [T, F, B, C]
    XT = np.zeros((T, F, BATCH, C + 1), dtype=np.float32)
    XT[:, :, :, 1:1 + C] = xr

    xdt_np = np.float16 if X_FP16 else np.float32
    in_maps1 = []
    for c in range(NCORES):
        sl = slice(c * CH, (c + 1) * CH)
        in_maps1.append({
            "wts": tw[sl].transpose(1, 0, 2).astype(xdt_np),  # [T, CH, 2T]
            "xt": XT[:, sl].astype(xdt_np),                   # [T, CH, B, C+1]
        })
    r1 = run_bass_kernel_spmd(nc1, in_maps1, core_ids=list(range(NCORES)))
    LAST_EXEC_NS["l1"] = r1.exec_time_ns
    ys = np.stack([r1.results[c]["y"] for c in range(NCORES)])  # [8, T, CH, B*C]
    ys = ys.reshape(NCORES, T, CH, BATCH, C)

    # ---- host mid: un-flip backward channels (time l = c*T + i reverses in
    # both i and c), then assemble partition-major [B, p, t, F] for launch 2
    ys[NCORES // 2:] = ys[NCORES // 2:, ::-1, :, :, ::-1]
    # [core, i, ch, b, c] -> [b, p=i, t=c, f=(core, ch)]
    ytd = np.ascontiguousarray(ys.transpose(3, 1, 4, 0, 2)).reshape(BATCH, T, NT, F)

    in_maps2 = []
    for c in range(NCORES):
        m = {"yt": ytd[c]}
        if apply_w:
            m["wv"] = ln_w.astype(np.float32).reshape(1, F)
        if apply_b:
            m["bv"] = ln_b.astype(np.float32).reshape(1, F)
        in_maps2.append(m)
    r2 = run_bass_kernel_spmd(nc2, in_maps2, core_ids=list(range(NCORES)))
    LAST_EXEC_NS["l2"] = r2.exec_time_ns
    outd = np.stack([r2.results[c]["out"] for c in range(NCORES)])  # [B, p, t, F]
    # [b, p, t, f] -> [b, f, l=t*T+p]
    out = np.ascontiguousarray(
        outd.transpose(0, 3, 2, 1).astype(np.float32)).reshape(BATCH, F, L)
    return out
